# revision 11
# baseline (speedup 1.0000x reference)
"""3x3 conv via 1D Winograd F(8,3) along W as full-height matmul bands on TRN2.

Full inputs: x [32, 128, 56, 56] f32, w [1152, 256] f32 (row = c*9 + kh*3 + kw).
Full output: [32, 256, 56, 56] f32. Data-parallel: 4 images per core, 8 cores.

The W-direction 3-tap conv is Winograd-transformed with m=8: each output
8-column tile consumes a 10-point window of x through the B^T data transform
(host-computed, 10 planes, nodes {0,±1/2,±3/4,±4/3,±2,inf}), contracted
against host-pre-transformed weights U = G g (10 planes x 3 vertical taps),
with the vertical taps accumulated in PSUM. Per (image, oc-half) each plane
is ONE matmul band of N = 56*7 = 392 columns (whole image height), so the
PE streams 3*10*392 cycles per 128*3136 outputs: 2.4x fewer columns than
direct convolution and near-zero per-matmul issue overhead.

PSUM: one plane per 2KB bank, 10 planes cycling 8 banks (bufs=1 tags); the
only PSUM readers are per-plane fp16 evacuation copies alternating between
ScalarE and VectorE, so bank reuse never stalls the matmul stream. The
device returns the ten m-planes in fp16; the host applies the exact 8x10
A^T inverse (small fraction of FLOPs) and interleaves.

Startup: weights stream on the sync HWDGE ring, data planes on the scalar
ring, fine-grained chunks first so plane p of image 0 lands just ahead of
its matmuls while warmup matmuls cover the preamble and HAM clock-gate ramp.
"""

import numpy as np

import concourse.bass as bass  # noqa: F401  (registers AP types)
import concourse.mybir as mybir
import concourse.tile as tile
from concourse import bacc, bass_utils

B, C, H, W = 32, 128, 56, 56
COUT = 256
NCORES = 8
BPC = B // NCORES  # images per core
M = 8  # winograd output tile size
NP = M + 2  # 10 winograd points/planes
WT = W // M  # 7 column tiles
HP = H + 2  # D rows: output row h needs D rows h..h+2 (x rows h-1..h+1)
R = H  # single full-height band
N = R * WT  # matmul free size per plane (392)
NODES = [0.0, 0.5, -0.5, 0.75, -0.75, 4.0 / 3.0, -4.0 / 3.0, 2.0, -2.0]
F32 = mybir.dt.float32
F16 = mybir.dt.float16
BF16 = mybir.dt.bfloat16
MOV = mybir.dt.float16
MOV_NP = np.float16


def _transforms():
    """Toom-Cook correlation transform (transposition principle).

    y = AT ((G g) . (BT d)) computes y_i = sum_k g_k d_{i+k} exactly:
    G/AT are node-power evaluations (+ leading-coeff rows for the inf
    node), BT = inv(E)^T with E the coefficient-evaluation matrix.
    """
    n, r, m = NP, 3, M
    a = np.array(NODES, np.float64)
    G = np.zeros((n, r))
    G[: n - 1] = a[:, None] ** np.arange(r)[None, :]
    G[n - 1, r - 1] = 1.0
    AT = np.zeros((m, n))
    AT[:, : n - 1] = a[None, :] ** np.arange(m)[:, None]
    AT[m - 1, n - 1] = 1.0
    E = np.zeros((n, n))
    E[: n - 1] = a[:, None] ** np.arange(n)[None, :]
    E[n - 1, n - 1] = 1.0
    BT = np.linalg.inv(E).T
    return (BT.astype(np.float32), G.astype(np.float32), AT.astype(np.float32))


BT32, G32, AT32 = _transforms()

_cached_nc = None


def _build():
    nc = bacc.Bacc(None, target_bir_lowering=False)
    # c-major so every DMA slice matches the SBUF tile's axis order
    d = nc.dram_tensor("d", [C, BPC, NP, HP, WT], MOV, kind="ExternalInput")
    # host pre-transformed weights: [oc_half, c, p, kh, 128]
    w = nc.dram_tensor("w", [2, C, NP, 3, 128], MOV, kind="ExternalInput")
    out = nc.dram_tensor("out", [BPC, COUT, NP, N], F16, kind="ExternalOutput")

    with tile.TileContext(nc) as tc:
        with (
            tc.tile_pool(name="wpool", bufs=1) as wpool,
            tc.tile_pool(name="dpool", bufs=1) as dpool,
            tc.tile_pool(name="opool", bufs=4) as opool,
            tc.tile_pool(name="pspool", bufs=1, space="PSUM") as pspool,
        ):
            # input DMAs: och0 weights alone on the sync ring (small, fast);
            # everything else on the scalar ring in exact consumption order
            # so no late chunk can starve an earlier-needed one (the SDMA
            # engines round-robin queues at packet granularity, so a big
            # transfer on one queue throttles the other queue's chunks)
            wbuf = wpool.tile([C, 2, NP, 3, 128], MOV)
            DD = dpool.tile([C, BPC, NP, HP, WT], MOV, tag="D")
            nc.sync.dma_start(wbuf[:, 0, 0], w[0, :, 0])
            nc.scalar.dma_start(DD[:, 0, 0], d[:, 0, 0])
            nc.sync.dma_start(wbuf[:, 0, 1], w[0, :, 1])
            nc.scalar.dma_start(DD[:, 0, 1], d[:, 0, 1])
            nc.sync.dma_start(wbuf[:, 0, 2:5], w[0, :, 2:5])
            nc.scalar.dma_start(DD[:, 0, 2:5], d[:, 0, 2:5])
            nc.sync.dma_start(wbuf[:, 0, 5:10], w[0, :, 5:10])
            nc.scalar.dma_start(DD[:, 0, 5:10], d[:, 0, 5:10])
            nc.scalar.dma_start(wbuf[:, 1, 0:5], w[1, :, 0:5])
            nc.scalar.dma_start(wbuf[:, 1, 5:10], w[1, :, 5:10])
            # D1-3 triggers are emitted inside the (b0, och0) loop below so
            # the scalar engine issues the first PSUM copies without delay

            # PE warmup: cover the preamble-to-first-data window and start
            # the HAM clock-gate busy window early
            NWARM = 4
            warm = wpool.tile([C, 448], BF16)
            nc.vector.memset(warm[:], 0.0)
            wps = pspool.tile([C, 2, 512], F32, tag="b3", name="warm")
            for i in range(NWARM):
                nc.tensor.matmul(wps[:16, 0, 0:448], warm[:, :16], warm[:],
                                 start=(i == 0), stop=(i == NWARM - 1))

            spp = 0  # global plane-pair index
            for b in range(BPC):
                for och in range(2):
                    OB = opool.tile([C, NP, N], F16, tag="ob",
                                    name=f"ob{b}{och}")
                    ocr = slice(och * 128, (och + 1) * 128)
                    last = b == BPC - 1 and och == 1
                    for pp in range(NP // 2):
                        # two planes per PSUM tile (adjacent banks), one
                        # evacuation copy per pair
                        ps = pspool.tile([C, 2, 512], F32, tag=f"b{spp % 4}",
                                         name=f"ps{spp}")
                        for sub in range(2):
                            p = 2 * pp + sub
                            for kh in range(3):
                                nc.tensor.matmul(
                                    ps[:, sub, 0:N],
                                    wbuf[:, och, p, kh, :],
                                    DD[:, b, p, kh : kh + R, :],
                                    start=(kh == 0),
                                    stop=(kh == 2),
                                )
                        dst = OB[:, 2 * pp : 2 * pp + 2]
                        if last and pp == NP // 2 - 1:
                            # final pair: split the copy across both engines
                            # and DMA per plane on both rings so the shortest
                            # possible chain trails the last matmul
                            nc.scalar.copy(out=OB[:, 2 * pp],
                                           in_=ps[:, 0, 0:N])
                            nc.sync.dma_start(out[b, ocr, 2 * pp],
                                              OB[:, 2 * pp])
                            nc.vector.tensor_copy(out=OB[:, 2 * pp + 1],
                                                  in_=ps[:, 1, 0:N])
                            nc.scalar.dma_start(out[b, ocr, 2 * pp + 1],
                                                OB[:, 2 * pp + 1])
                        elif spp % 2 == 0:
                            nc.scalar.copy(out=dst, in_=ps[:, :, 0:N])
                        else:
                            nc.vector.tensor_copy(out=dst, in_=ps[:, :, 0:N])
                        if b == 0 and och == 0 and pp >= 2:
                            # rest-of-input triggers sit here so the scalar
                            # engine issues the first copies without delay
                            bb = pp - 1
                            nc.scalar.dma_start(DD[:, bb], d[:, bb])
                        if last and pp < NP // 2 - 1:
                            # drain the last oc-half in plane pairs across
                            # both HWDGE rings so only ~100KB trails the end
                            ring = nc.sync if spp % 2 == 0 else nc.scalar
                            ring.dma_start(out[b, ocr, 2 * pp : 2 * pp + 2],
                                           OB[:, 2 * pp : 2 * pp + 2])
                        spp += 1
                    if not last:
                        ring = nc.sync if och == 0 else nc.scalar
                        ring.dma_start(out[b, ocr], OB[:])
    nc.compile()
    return nc


def _get_nc():
    global _cached_nc
    if _cached_nc is None:
        _cached_nc = _build()
    return _cached_nc


def _host_weights(w):
    """w [1152, 256] f32 -> [oc_half, c, p, kh, 128] fp16 G-transformed."""
    g = np.asarray(w, dtype=np.float32).reshape(C, 3, 3, COUT)
    U = np.einsum("pk,chko->pcho", G32, g)  # [NP, c, kh, oc]
    return np.ascontiguousarray(
        U.reshape(NP, C, 3, 2, 128).transpose(3, 1, 0, 2, 4)
    ).astype(MOV_NP)


def _host_fwd(x):
    """x [B, C, H, W] f32 -> D [C, B, NP, 58, WT] fp16 (B^T transform)."""
    x = np.asarray(x, dtype=np.float32)
    xw = np.pad(x, ((0, 0), (0, 0), (0, 0), (1, 1)))
    win = np.stack([xw[..., M * t : M * t + NP] for t in range(WT)], axis=-2)
    # win: [B, C, H, WT, NP]
    D = np.zeros((C, B, NP, HP, WT), MOV_NP)
    D[:, :, :, 1 : H + 1, :] = np.einsum("pj,bchtj->cbpht", BT32, win)
    return D


def run(x, w, trace=False, **spmd_kwargs):
    nc = _get_nc()
    dfull = _host_fwd(x)
    w2 = _host_weights(w)
    in_maps = [
        {"d": np.ascontiguousarray(dfull[:, i * BPC : (i + 1) * BPC]), "w": w2}
        for i in range(NCORES)
    ]
    res = bass_utils.run_bass_kernel_spmd(
        nc, in_maps, core_ids=list(range(NCORES)), trace=trace, **spmd_kwargs
    )
    # dev out m-planes [BPC, 256, NP, 392] -> A^T inverse -> full output
    m = np.concatenate([r["out"] for r in res.results], axis=0).astype(np.float32)
    m = m.reshape(B, COUT, NP, R, WT)
    y = np.einsum("jp,bopht->bohtj", AT32, m)  # [B, 256, H, WT, M]
    full = np.ascontiguousarray(y.reshape(B, COUT, H, W))
    return full, res


def kernel(x, w):
    return run(x, w)[0]


# revision 13
# speedup vs baseline: 1.0574x; 1.0574x over previous
"""3x3 conv via 1D Winograd F(8,3) along W as full-height matmul bands on TRN2.

Full inputs: x [32, 128, 56, 56] f32, w [1152, 256] f32 (row = c*9 + kh*3 + kw).
Full output: [32, 256, 56, 56] f32. Data-parallel: 4 images per core, 8 cores.

The W-direction 3-tap conv is Winograd-transformed with m=8: each output
8-column tile consumes a 10-point window of x through the B^T data transform
(host-computed, 10 planes, nodes {0,±1/2,±3/4,±4/3,±2,inf}), contracted
against host-pre-transformed weights U = G g (10 planes x 3 vertical taps),
with the vertical taps accumulated in PSUM. Per (image, oc-half) each plane
is ONE matmul band of N = 56*7 = 392 columns (whole image height), so the
PE streams 3*10*392 cycles per 128*3136 outputs: 2.4x fewer columns than
direct convolution and near-zero per-matmul issue overhead.

PSUM: one plane per 2KB bank, 10 planes cycling 8 banks (bufs=1 tags); the
only PSUM readers are per-plane fp16 evacuation copies alternating between
ScalarE and VectorE, so bank reuse never stalls the matmul stream. The
device returns the ten m-planes in fp16; the host applies the exact 8x10
A^T inverse (small fraction of FLOPs) and interleaves.

Startup: weights stream on the sync HWDGE ring, data planes on the scalar
ring, fine-grained chunks first so plane p of image 0 lands just ahead of
its matmuls while warmup matmuls cover the preamble and HAM clock-gate ramp.
"""

import numpy as np

import concourse.bass as bass  # noqa: F401  (registers AP types)
import concourse.mybir as mybir
import concourse.tile as tile
from concourse import bacc, bass_utils

B, C, H, W = 32, 128, 56, 56
COUT = 256
NCORES = 8
BPC = B // NCORES  # images per core
M = 8  # winograd output tile size
NP = M + 2  # 10 winograd points/planes
WT = W // M  # 7 column tiles
HP = H + 2  # D rows: output row h needs D rows h..h+2 (x rows h-1..h+1)
R = H  # single full-height band
N = R * WT  # matmul free size per plane (392)
NODES = [0.0, 0.5, -0.5, 0.75, -0.75, 4.0 / 3.0, -4.0 / 3.0, 2.0, -2.0]
F32 = mybir.dt.float32
F16 = mybir.dt.float16
BF16 = mybir.dt.bfloat16
MOV = mybir.dt.float16
MOV_NP = np.float16


def _transforms():
    """Toom-Cook correlation transform (transposition principle).

    y = AT ((G g) . (BT d)) computes y_i = sum_k g_k d_{i+k} exactly:
    G/AT are node-power evaluations (+ leading-coeff rows for the inf
    node), BT = inv(E)^T with E the coefficient-evaluation matrix.
    """
    n, r, m = NP, 3, M
    a = np.array(NODES, np.float64)
    G = np.zeros((n, r))
    G[: n - 1] = a[:, None] ** np.arange(r)[None, :]
    G[n - 1, r - 1] = 1.0
    AT = np.zeros((m, n))
    AT[:, : n - 1] = a[None, :] ** np.arange(m)[:, None]
    AT[m - 1, n - 1] = 1.0
    E = np.zeros((n, n))
    E[: n - 1] = a[:, None] ** np.arange(n)[None, :]
    E[n - 1, n - 1] = 1.0
    BT = np.linalg.inv(E).T
    return (BT.astype(np.float32), G.astype(np.float32), AT.astype(np.float32))


BT32, G32, AT32 = _transforms()

_cached_nc = None


def _build():
    nc = bacc.Bacc(None, target_bir_lowering=False)
    # c-major so every DMA slice matches the SBUF tile's axis order
    d = nc.dram_tensor("d", [C, BPC, NP, HP, WT], MOV, kind="ExternalInput")
    # host pre-transformed weights: [oc_half, c, p, kh, 128]
    w = nc.dram_tensor("w", [2, C, NP, 3, 128], MOV, kind="ExternalInput")
    out = nc.dram_tensor("out", [BPC, COUT, NP, N], F16, kind="ExternalOutput")

    with tile.TileContext(nc) as tc:
        with (
            tc.tile_pool(name="wpool", bufs=1) as wpool,
            tc.tile_pool(name="dpool", bufs=1) as dpool,
            tc.tile_pool(name="opool", bufs=4) as opool,
            tc.tile_pool(name="pspool", bufs=1, space="PSUM") as pspool,
        ):
            # input DMAs: och0 weights alone on the sync ring (small, fast);
            # everything else on the scalar ring in exact consumption order
            # so no late chunk can starve an earlier-needed one (the SDMA
            # engines round-robin queues at packet granularity, so a big
            # transfer on one queue throttles the other queue's chunks)
            wbuf = wpool.tile([C, 2, NP, 3, 128], MOV)
            DD = dpool.tile([C, BPC, NP, HP, WT], MOV, tag="D")
            nc.sync.dma_start(wbuf[:, 0, 0], w[0, :, 0])
            nc.scalar.dma_start(DD[:, 0, 0], d[:, 0, 0])
            nc.sync.dma_start(wbuf[:, 0, 1], w[0, :, 1])
            nc.scalar.dma_start(DD[:, 0, 1], d[:, 0, 1])
            nc.sync.dma_start(wbuf[:, 0, 2:5], w[0, :, 2:5])
            nc.scalar.dma_start(DD[:, 0, 2:5], d[:, 0, 2:5])
            nc.sync.dma_start(wbuf[:, 0, 5:10], w[0, :, 5:10])
            nc.scalar.dma_start(DD[:, 0, 5:10], d[:, 0, 5:10])
            nc.scalar.dma_start(wbuf[:, 1, 0:5], w[1, :, 0:5])
            nc.scalar.dma_start(wbuf[:, 1, 5:10], w[1, :, 5:10])
            # D1-3 triggers are emitted inside the (b0, och0) loop below so
            # the scalar engine issues the first PSUM copies without delay

            # PE warmup: cover the preamble-to-first-data window and start
            # the HAM clock-gate busy window early
            NWARM = 6
            warm = wpool.tile([C, 448], BF16)
            nc.vector.memset(warm[:], 0.0)
            wps = pspool.tile([C, 2, 512], F32, tag="b3", name="warm")
            for i in range(NWARM):
                nc.tensor.matmul(wps[:16, 0, 0:448], warm[:, :16], warm[:],
                                 start=(i == 0), stop=(i == NWARM - 1))

            def bridge(tag, k, n=2):
                # filler matmuls between image-0 plane pairs: keep the PE
                # busy across input-chunk boundaries so the HAM busy window
                # keeps accumulating even if the next chunk is late. They
                # write the upper scratch of the next pair's psum tile,
                # which the real matmuls' start=True then reclaims.
                bps = pspool.tile([C, 2, 512], F32, tag=tag, name=f"br{k}")
                for i in range(n):
                    nc.tensor.matmul(bps[:16, 0, 0:448], warm[:, :16],
                                     warm[:], start=(i == 0),
                                     stop=(i == n - 1))

            spp = 0  # global plane-pair index
            for b in range(BPC):
                for och in range(2):
                    OB = opool.tile([C, NP, N], F16, tag="ob",
                                    name=f"ob{b}{och}")
                    ocr = slice(och * 128, (och + 1) * 128)
                    last = b == BPC - 1 and och == 1
                    for pp in range(NP // 2):
                        # two planes per PSUM tile (adjacent banks), one
                        # evacuation copy per pair
                        ps = pspool.tile([C, 2, 512], F32, tag=f"b{spp % 4}",
                                         name=f"ps{spp}")
                        for sub in range(2):
                            p = 2 * pp + sub
                            for kh in range(3):
                                nc.tensor.matmul(
                                    ps[:, sub, 0:N],
                                    wbuf[:, och, p, kh, :],
                                    DD[:, b, p, kh : kh + R, :],
                                    start=(kh == 0),
                                    stop=(kh == 2),
                                )
                        dst = OB[:, 2 * pp : 2 * pp + 2]
                        if last and pp == NP // 2 - 1:
                            # final pair: split the copy across both engines
                            # and DMA per plane on both rings so the shortest
                            # possible chain trails the last matmul
                            nc.scalar.copy(out=OB[:, 2 * pp],
                                           in_=ps[:, 0, 0:N])
                            nc.sync.dma_start(out[b, ocr, 2 * pp],
                                              OB[:, 2 * pp])
                            nc.vector.tensor_copy(out=OB[:, 2 * pp + 1],
                                                  in_=ps[:, 1, 0:N])
                            nc.scalar.dma_start(out[b, ocr, 2 * pp + 1],
                                                OB[:, 2 * pp + 1])
                        elif spp % 2 == 0:
                            nc.scalar.copy(out=dst, in_=ps[:, :, 0:N])
                        else:
                            nc.vector.tensor_copy(out=dst, in_=ps[:, :, 0:N])
                        if b == 0 and och == 0 and pp >= 2:
                            # rest-of-input triggers sit here so the scalar
                            # engine issues the first copies without delay
                            bb = pp - 1
                            nc.scalar.dma_start(DD[:, bb], d[:, bb])
                        if b == 0 and och == 0 and pp < 3:
                            bridge(f"b{(spp + 1) % 4}", spp)
                        if last and pp < NP // 2 - 1:
                            # drain the last oc-half in plane pairs across
                            # both HWDGE rings so only ~100KB trails the end
                            ring = nc.sync if spp % 2 == 0 else nc.scalar
                            ring.dma_start(out[b, ocr, 2 * pp : 2 * pp + 2],
                                           OB[:, 2 * pp : 2 * pp + 2])
                        spp += 1
                    if not last:
                        ring = nc.sync if och == 0 else nc.scalar
                        ring.dma_start(out[b, ocr], OB[:])
    nc.compile()
    return nc


def _get_nc():
    global _cached_nc
    if _cached_nc is None:
        _cached_nc = _build()
    return _cached_nc


def _host_weights(w):
    """w [1152, 256] f32 -> [oc_half, c, p, kh, 128] fp16 G-transformed."""
    g = np.asarray(w, dtype=np.float32).reshape(C, 3, 3, COUT)
    U = np.einsum("pk,chko->pcho", G32, g)  # [NP, c, kh, oc]
    return np.ascontiguousarray(
        U.reshape(NP, C, 3, 2, 128).transpose(3, 1, 0, 2, 4)
    ).astype(MOV_NP)


def _host_fwd(x):
    """x [B, C, H, W] f32 -> D [C, B, NP, 58, WT] fp16 (B^T transform)."""
    x = np.asarray(x, dtype=np.float32)
    xw = np.pad(x, ((0, 0), (0, 0), (0, 0), (1, 1)))
    win = np.stack([xw[..., M * t : M * t + NP] for t in range(WT)], axis=-2)
    # win: [B, C, H, WT, NP]
    D = np.zeros((C, B, NP, HP, WT), MOV_NP)
    D[:, :, :, 1 : H + 1, :] = np.einsum("pj,bchtj->cbpht", BT32, win)
    return D


def run(x, w, trace=False, **spmd_kwargs):
    nc = _get_nc()
    dfull = _host_fwd(x)
    w2 = _host_weights(w)
    in_maps = [
        {"d": np.ascontiguousarray(dfull[:, i * BPC : (i + 1) * BPC]), "w": w2}
        for i in range(NCORES)
    ]
    res = bass_utils.run_bass_kernel_spmd(
        nc, in_maps, core_ids=list(range(NCORES)), trace=trace, **spmd_kwargs
    )
    # dev out m-planes [BPC, 256, NP, 392] -> A^T inverse -> full output
    m = np.concatenate([r["out"] for r in res.results], axis=0).astype(np.float32)
    m = m.reshape(B, COUT, NP, R, WT)
    y = np.einsum("jp,bopht->bohtj", AT32, m)  # [B, 256, H, WT, M]
    full = np.ascontiguousarray(y.reshape(B, COUT, H, W))
    return full, res


def kernel(x, w):
    return run(x, w)[0]
